# revision 1
# baseline (speedup 1.0000x reference)
"""Causal multi-head attention on 8 Trainium2 NeuronCores.

Sharding: Megatron-style tensor parallelism over heads. Each of the 8
cores computes 2 of the 16 heads end-to-end:
  - column-parallel Q/K/V projections (each core owns a 256-feature slice
    of wq/wk/wv),
  - per-head causal attention entirely on-core,
  - row-parallel output projection producing a partial [B*S, D] output.
The 8 partials are summed on the host (the "all-reduce") and bo added.

Device layout notes:
  - All matmul operands are bf16 (the PE streams bf16 moving operands at
    ~2x the fp32/fp32r rate); accumulation is fp32 in PSUM, softmax
    statistics and the partial outputs stay fp32.
  - Activations are kept feature-major (transposed): qT/kT are [hd, S]
    per head, scores are computed transposed ([k, q]) so the exp'd
    probabilities feed the PV matmul directly as the moving operand and
    the softmax denominator comes from a ones-matmul — the kernel
    contains zero on-chip transposes.
  - Causality is exploited at block granularity: upper-triangle score
    blocks are never computed; diagonal blocks get a static additive
    mask; partially-causal blocks are column-trimmed at 128 granularity.
"""

import math

import numpy as np

B = 2
S = 2048
D = 2048
H = 16
HD = 128  # head dim
N_CORES = 8
H_LOC = H // N_CORES       # 2 heads per core
F_LOC = H_LOC * HD         # 256 local features per core
KT = D // 128              # 16 contraction tiles
CHUNK = 512                # token chunk (matmul moving dim)
NCH = S // CHUNK           # 4 chunks per batch
TT = S // 128              # 16 token tiles per batch

_CACHE = {}


MM_DTYPE = "bf16"  # "bf16" or "f32r" — dtype of all matmul operands
STRUCTURE = "v1"   # "v1": per-batch QKV->attn+outproj; "v2": cross-batch pipeline


def _build(reps=None, mm_dtype=None):
    import concourse.mybir as mybir
    import concourse.tile as tile
    from concourse import bacc

    F32 = mybir.dt.float32
    # MDT is the dtype of every matmul operand (bf16 by default: the PE
    # streams bf16 at 2 cols/cycle vs fp32r's effective half rate).
    MDT = (mybir.dt.bfloat16 if (mm_dtype or MM_DTYPE) == "bf16"
           else mybir.dt.float32r)
    ADD = mybir.AluOpType.add
    MULT = mybir.AluOpType.mult
    EXP = mybir.ActivationFunctionType.Exp
    INV_SQRT_HD = 1.0 / math.sqrt(HD)

    nc = bacc.Bacc("TRN2", target_bir_lowering=False, debug=False,
                   num_devices=N_CORES)

    xT_d = nc.dram_tensor("xT", [D, B * S], MDT, kind="ExternalInput")
    wqT_d = nc.dram_tensor("wqT", [D, F_LOC], MDT, kind="ExternalInput")
    wkT_d = nc.dram_tensor("wkT", [D, F_LOC], MDT, kind="ExternalInput")
    wvT_d = nc.dram_tensor("wvT", [D, F_LOC], MDT, kind="ExternalInput")
    woT_d = nc.dram_tensor("woT", [F_LOC, D], MDT, kind="ExternalInput")
    bq_d = nc.dram_tensor("bq2", [HD, H_LOC], F32, kind="ExternalInput")
    bk_d = nc.dram_tensor("bk2", [HD, H_LOC], F32, kind="ExternalInput")
    bv_d = nc.dram_tensor("bvr", [128, F_LOC], F32, kind="ExternalInput")
    ones_d = nc.dram_tensor("ones", [128, 128], MDT, kind="ExternalInput")
    y_d = nc.dram_tensor("y", [B * S, D], F32, kind="ExternalOutput")

    with tile.TileContext(nc) as tc:
        cpool = tc.alloc_tile_pool(name="const", bufs=1)
        wpool = tc.alloc_tile_pool(name="w", bufs=1)
        xkpool = tc.alloc_tile_pool(name="xk", bufs=4)
        actpool = tc.alloc_tile_pool(name="act", bufs=8)
        ypool = tc.alloc_tile_pool(name="y", bufs=4)
        ripool = tc.alloc_tile_pool(name="ri", bufs=2)
        # one deep shared accumulator pool (QKV / scores / PV / rowsum)
        # plus a small dedicated pool for the output projection
        psq = tc.alloc_tile_pool(name="ps", bufs=5, space="PSUM")
        psa = psr = psq
        pso = tc.alloc_tile_pool(name="pso", bufs=3, space="PSUM")
        qtag = "ps"

        # --- loads; DMA queue order is deliberate (wq + first x chunks
        # first so the projection matmuls start a few us in) ---
        def load_w(nm, dram):
            w_t = wpool.tile([128, KT * F_LOC], MDT, tag=nm)
            nc.sync.dma_start(
                w_t[:].rearrange("p (k f) -> p k f", k=KT),
                dram.ap().rearrange("(k p) f -> p k f", p=128),
            )
            return w_t

        def load_x(b, c, split=1):
            x_t = xkpool.tile([128, KT * CHUNK], MDT, tag="xk")
            col0 = b * S + c * CHUNK
            kstep = KT // split
            for s in range(split):
                k0 = s * kstep
                nc.sync.dma_start(
                    x_t[:, k0 * CHUNK:(k0 + kstep) * CHUNK]
                        .rearrange("p (k f) -> p k f", k=kstep),
                    xT_d.ap()[k0 * 128:(k0 + kstep) * 128,
                              col0:col0 + CHUNK]
                        .rearrange("(k p) f -> p k f", p=128),
                )
            return x_t

        w_ts = {"wq": load_w("wq", wqT_d)}
        x_first = load_x(0, 0, split=4)
        w_ts["wk"] = load_w("wk", wkT_d)
        w_ts["wv"] = load_w("wv", wvT_d)
        x_second = load_x(0, 1, split=2)
        bq_t = cpool.tile([HD, H_LOC], F32, tag="bq")
        bk_t = cpool.tile([HD, H_LOC], F32, tag="bk")
        bv_t = cpool.tile([128, F_LOC], F32, tag="bv")
        nc.sync.dma_start(bq_t[:], bq_d.ap())
        nc.sync.dma_start(bk_t[:], bk_d.ap())
        nc.sync.dma_start(bv_t[:], bv_d.ap())
        ones128 = cpool.tile([128, 128], MDT, tag="ones128")
        nc.sync.dma_start(ones128[:], ones_d.ap())
        # warm the ACT Exp table during the QKV phase so the table load
        # doesn't land on the first attention chunk
        warm_t = cpool.tile([128, 1], F32, tag="warm")
        nc.scalar.activation(warm_t[:], bq_t[:, 0:1], EXP,
                             bias=0.0, scale=1.0)
        maskT = cpool.tile([128, 128], F32, tag="maskT")
        nc.gpsimd.memset(maskT[:], 0.0)
        # transposed causal mask: keep (0) where k_part <= q_free else -1e9
        nc.gpsimd.affine_select(
            out=maskT[:], in_=maskT[:],
            compare_op=mybir.AluOpType.is_ge,
            fill=-1e9, base=0, pattern=[[1, 128]], channel_multiplier=-1,
        )

        def qkv_chunk(x_t, c, acts):
            """Q/K/V projections for one 512-token chunk."""
            qT_t, kT_t, v_t, _ = acts
            for nm, dst, bias in (("wq", qT_t, bq_t), ("wk", kT_t, bk_t)):
                for h in range(H_LOC):
                    q_ps = psq.tile([128, CHUNK], F32, tag=qtag)
                    for k in range(KT):
                        nc.tensor.matmul(
                            q_ps[:],
                            w_ts[nm][:, k * F_LOC + h * HD:
                                     k * F_LOC + (h + 1) * HD],
                            x_t[:, k * CHUNK:(k + 1) * CHUNK],
                            start=(k == 0), stop=(k == KT - 1),
                        )
                    nc.vector.tensor_scalar_add(
                        dst[:, h * S + c * CHUNK: h * S + (c + 1) * CHUNK],
                        q_ps[:], bias[:, h:h + 1])
            for t4 in range(CHUNK // 128):
                tt = c * (CHUNK // 128) + t4
                v_ps = psq.tile([128, CHUNK], F32, tag=qtag)
                for k in range(KT):
                    nc.tensor.matmul(
                        v_ps[:, 0:F_LOC],
                        x_t[:, k * CHUNK + t4 * 128:
                            k * CHUNK + (t4 + 1) * 128],
                        w_ts["wv"][:, k * F_LOC:(k + 1) * F_LOC],
                        start=(k == 0), stop=(k == KT - 1),
                    )
                # bias folded into the PSUM->SBUF copy (bv broadcast across
                # partitions, prepared on the host)
                nc.vector.tensor_tensor(
                    v_t[:, tt * F_LOC:(tt + 1) * F_LOC],
                    v_ps[:, 0:F_LOC], bv_t[:], ADD)

        def attn_chunk(c, acts):
            """Causal attention for one 512-query chunk, both heads."""
            qT_t, kT_t, v_t, attnT_t = acts
            nki = 4 * c + 4
            for h in range(H_LOC):
                e_t = xkpool.tile([128, KT * CHUNK], MDT, tag="xk")
                q0 = h * S + c * CHUNK
                # scoresT blocks + exp (transposed layout: [k, q])
                for ki in range(nki):
                    r = ki - 4 * c
                    trim = 128 * r if r > 0 else 0
                    ncol = CHUNK - trim
                    s_ps = psa.tile([128, CHUNK], F32, tag=qtag)
                    nc.tensor.matmul(
                        s_ps[:, 0:ncol],
                        kT_t[:, h * S + ki * 128: h * S + (ki + 1) * 128],
                        qT_t[:, q0 + trim: q0 + CHUNK],
                        start=True, stop=True,
                    )
                    if ki >= 4 * c:  # diagonal 128x128 needs the mask
                        nc.vector.tensor_tensor(
                            s_ps[:, 0:128], s_ps[:, 0:128], maskT[:], ADD)
                    nc.scalar.activation(
                        e_t[:, ki * CHUNK + trim:(ki + 1) * CHUNK],
                        s_ps[:, 0:ncol], EXP, bias=0.0, scale=INV_SQRT_HD)
                # PV and rowsum accumulations over ki (PE)
                at_ps = psr.tile([128, CHUNK], F32, tag=qtag)
                rs_ps = psr.tile([128, CHUNK], F32, tag=qtag)
                for ki in range(nki):
                    r = ki - 4 * c
                    trim = 128 * r if r > 0 else 0
                    nc.tensor.matmul(
                        at_ps[:, trim:CHUNK],
                        v_t[:, ki * F_LOC + h * HD:
                            ki * F_LOC + (h + 1) * HD],
                        e_t[:, ki * CHUNK + trim:(ki + 1) * CHUNK],
                        start=(ki == 0), stop=(ki == nki - 1),
                    )
                for ki in range(nki):
                    r = ki - 4 * c
                    trim = 128 * r if r > 0 else 0
                    nc.tensor.matmul(
                        rs_ps[:, trim:CHUNK],
                        ones128[:],
                        e_t[:, ki * CHUNK + trim:(ki + 1) * CHUNK],
                        start=(ki == 0), stop=(ki == nki - 1),
                    )
                ri_t = ripool.tile([128, CHUNK], F32, tag="ri")
                nc.vector.reciprocal(ri_t[:], rs_ps[:])
                nc.vector.tensor_tensor(
                    attnT_t[:, q0: q0 + CHUNK],
                    at_ps[:], ri_t[:], MULT)

        def outproj_chunk(b, c, acts, wo_t):
            """Output projection + y writeback for one chunk's tokens."""
            attnT_t = acts[3]
            for t4 in range(CHUNK // 128):
                tt = c * (CHUNK // 128) + t4
                for oc in range(D // CHUNK):
                    o_ps = pso.tile([128, CHUNK], F32, tag="pso")
                    for h in range(H_LOC):
                        nc.tensor.matmul(
                            o_ps[:],
                            attnT_t[:, h * S + tt * 128:
                                    h * S + (tt + 1) * 128],
                            wo_t[:, h * D + oc * CHUNK:
                                 h * D + (oc + 1) * CHUNK],
                            start=(h == 0), stop=(h == H_LOC - 1),
                        )
                    y_t = ypool.tile([128, CHUNK], F32, tag="y")
                    nc.vector.tensor_copy(y_t[:], o_ps[:])
                    row0 = b * S + tt * 128
                    nc.sync.dma_start(
                        y_d.ap()[row0:row0 + 128,
                                 oc * CHUNK:(oc + 1) * CHUNK], y_t[:])

        def new_acts():
            qT_t = actpool.tile([128, H_LOC * S], MDT, tag="act")
            kT_t = actpool.tile([128, H_LOC * S], MDT, tag="act")
            v_t = actpool.tile([128, TT * F_LOC], MDT, tag="act")
            attnT_t = actpool.tile([128, H_LOC * S], MDT, tag="act")
            return (qT_t, kT_t, v_t, attnT_t)

        def load_wo():
            # woT [F_LOC, D] -> [128, H_LOC*D]; deferred load so the DMA
            # queue prioritizes x chunks during warmup
            wo_t = wpool.tile([128, H_LOC * D], MDT, tag="wo")
            nc.sync.dma_start(
                wo_t[:].rearrange("p (h f) -> p h f", h=H_LOC),
                woT_d.ap().rearrange("(h p) f -> p h f", p=128),
            )
            return wo_t

        def emit_body_v2(first_iter=True):
            # phase 1: QKV(b0) — PE-bound, streams x(b0)
            acts0 = new_acts()
            for c in range(NCH):
                if first_iter and c == 0:
                    x_t = x_first
                elif first_iter and c == 1:
                    x_t = x_second
                else:
                    x_t = load_x(0, c, split=(4 if c == 0 else 1))
                qkv_chunk(x_t, c, acts0)

            wo_t = load_wo()

            # phase 2: QKV(b1) [PE-bound] interleaved with attention(b0)
            # [ACT-bound] + outproj(b0) [DVE/DMA-bound]
            acts1 = new_acts()
            for c in range(NCH):
                qkv_chunk(load_x(1, c), c, acts1)
                attn_chunk(c, acts0)
                outproj_chunk(0, c, acts0, wo_t)

            # phase 3: attention(b1) + outproj(b1)
            for c in range(NCH):
                attn_chunk(c, acts1)
                outproj_chunk(1, c, acts1, wo_t)

        def emit_body_v1(first_iter=True):
            wo_t = None
            for b in range(B):
                acts = new_acts()
                for c in range(NCH):
                    if first_iter and b == 0 and c == 0:
                        x_t = x_first
                    elif first_iter and b == 0 and c == 1:
                        x_t = x_second
                    else:
                        x_t = load_x(b, c,
                                     split=(4 if (b == 0 and c == 0) else 1))
                    if wo_t is None and c == NCH - 1:
                        wo_t = load_wo()
                    qkv_chunk(x_t, c, acts)
                for c in range(NCH):
                    attn_chunk(c, acts)
                    outproj_chunk(b, c, acts, wo_t)

        emit_body = emit_body_v2 if STRUCTURE == "v2" else emit_body_v1

        if reps is None:
            emit_body()
        else:
            with tc.For_i(0, reps, 1):
                emit_body(first_iter=False)

        pools = [pso, psq, ripool, ypool, actpool, xkpool,
                 wpool, cpool]
        seen = set()
        for p in pools:
            if id(p) not in seen:
                seen.add(id(p))
                p.release()

    nc.compile()
    return nc


def _get_nc(reps=None, mm_dtype=None):
    key = ("nc", reps, mm_dtype or MM_DTYPE)
    if key not in _CACHE:
        _CACHE[key] = _build(reps, mm_dtype)
    return _CACHE[key]


def _mm_np(a):
    """Cast a host array to the matmul operand dtype."""
    if MM_DTYPE == "bf16":
        import ml_dtypes
        return np.ascontiguousarray(a).astype(ml_dtypes.bfloat16)
    return np.ascontiguousarray(a).astype(np.float32)


def make_in_maps(x, wq, bq, wk, bk, wv, bv, wo):
    x = np.asarray(x, dtype=np.float32)
    xT = _mm_np(x.reshape(B * S, D).T)  # [D, B*S]

    in_maps = []
    for i in range(N_CORES):
        fs = slice(i * F_LOC, (i + 1) * F_LOC)
        in_maps.append({
            "xT": xT,
            "wqT": _mm_np(np.asarray(wq)[fs, :].T),
            "wkT": _mm_np(np.asarray(wk)[fs, :].T),
            "wvT": _mm_np(np.asarray(wv)[fs, :].T),
            "woT": _mm_np(np.asarray(wo)[:, fs].T),
            "bq2": np.ascontiguousarray(
                np.asarray(bq)[fs].reshape(H_LOC, HD).T),
            "bk2": np.ascontiguousarray(
                np.asarray(bk)[fs].reshape(H_LOC, HD).T),
            "bvr": np.ascontiguousarray(np.broadcast_to(
                np.asarray(bv, dtype=np.float32)[fs][None, :], (128, F_LOC))),
            "ones": _mm_np(np.ones((128, 128), dtype=np.float32)),
        })
    return in_maps


def kernel(x, wq, bq, wk, bk, wv, bv, wo, bo):
    from concourse.bass_utils import run_bass_kernel_spmd

    nc = _get_nc()
    in_maps = make_in_maps(x, wq, bq, wk, bk, wv, bv, wo)
    res = run_bass_kernel_spmd(nc, in_maps, core_ids=list(range(N_CORES)),
                               trace=False)
    y = np.zeros((B * S, D), dtype=np.float32)
    for i in range(N_CORES):
        y += res.results[i]["y"]
    y += np.asarray(bo, dtype=np.float32)[None, :]
    return y.reshape(B, S, D)

